# revision 7
# baseline (speedup 1.0000x reference)
"""Multi-head causal attention (B=2, S=2048, D=1024, H=16) on 8 TRN2 NeuronCores.

Sharding: core c handles batch b = c//4 and head-group g = c%4 (4 heads, 256 dims).
Each core computes Q/K/V projections for its head group from x[b], runs causal
attention per head, and applies its 256 rows of Wo, producing a partial [S, D]
output. The host sums the 4 head-group partials per batch.

Device algorithm (per core), all matmuls in float32r (full-rate, ~tf32):
  qT/kT = Wq_g^T @ x^T, stored [64*2, pair, S] (head dims on partitions)
  v     = x @ Wv_g, stored per 128-seq block with an appended ones column
  per head, per 512-wide i-chunk:
    S^T[j,i] strips via matmul(lhsT=kT_block, rhs=qT_chunk)   (K=64, row-paired)
    P~^T = exp(scale * S^T)  (ScalarE, batched over 3 strips), causal-masked
    O'^T[65, i] += V'_j^T @ P~^T_j   (PSUM accumulate; row 64 = softmax denom)
    O^T = O'^T[0:64] * recip(O'^T[64]) (denom broadcast via K=1 matmul)
  y = O @ Wo_g (lhsT = O^T tiles), DMA out.
"""

import os

import numpy as np

import concourse.bass as bass
import concourse.mybir as mybir
import concourse.tile as tile
from concourse.bass_utils import run_bass_kernel_spmd
from concourse.masks import make_upper_triangular

F32 = mybir.dt.float32
F32R = mybir.dt.float32r

B, S, D, H = 2, 2048, 1024, 16
HD = 64                     # head dim
GH = 4                      # heads per core
GC = GH * HD                # 256 projection cols per core
P = 128
KD = D // P                 # 8 contraction chunks for projections
NSB = S // P                # 16 seq blocks
CHW = 512                   # i-chunk width
NCH = S // CHW              # 4 i-chunks
SCALE = HD ** -0.5

_NC_CACHE = None
LAST_RESULTS = None         # BassKernelResults of the most recent run (for test.py)


def _r(ap):
    return ap.bitcast(F32R)


def _emit(tc):
    nc = tc.nc
    xT = nc.dram_tensor("xT", [D, S], F32, kind="ExternalInput")
    wq = nc.dram_tensor("wq", [D, GC], F32, kind="ExternalInput")
    wk = nc.dram_tensor("wk", [D, GC], F32, kind="ExternalInput")
    wv = nc.dram_tensor("wv", [D, GC], F32, kind="ExternalInput")
    wo = nc.dram_tensor("wo", [GC, D], F32, kind="ExternalInput")
    y = nc.dram_tensor("y", [S, D], F32, kind="ExternalOutput")

    xT_t = xT[:].rearrange("(o p) s -> p o s", p=P)      # [128, 8, S]
    wq_t = wq[:].rearrange("(o p) c -> p o c", p=P)      # [128, 8, 256]
    wk_t = wk[:].rearrange("(o p) c -> p o c", p=P)
    wv_t = wv[:].rearrange("(o p) c -> p o c", p=P)
    wo_t = wo[:].rearrange("(o p) n -> p o n", p=P)      # [128, 2, 1024]

    from contextlib import ExitStack

    with ExitStack() as top:
        persist = top.enter_context(tc.tile_pool(name="persist", bufs=1))

        trimask = persist.tile([P, P], F32)              # 1.0 where j<=i else 0
        make_upper_triangular(nc, trimask, val=1.0, diag=True)
        ones_f32 = persist.tile([P, HD], F32)
        nc.vector.memset(ones_f32, 1.0)
        zeros_f32 = persist.tile([P, 3 * P], F32)
        nc.vector.memset(zeros_f32, 0.0)
        ones_sb = persist.tile([P, HD], F32R)
        nc.vector.tensor_copy(out=ones_sb, in_=ones_f32)

        wq_sb = persist.tile([P, KD, GC], F32R)
        wk_sb = persist.tile([P, KD, GC], F32R)
        wv_sb = persist.tile([P, KD, GC], F32R)
        wo_sb = persist.tile([P, 2, D], F32R)
        nc.sync.dma_start(out=wq_sb, in_=wq_t.bitcast(F32R))
        nc.sync.dma_start(out=wk_sb, in_=wk_t.bitcast(F32R))
        nc.sync.dma_start(out=wv_sb, in_=wv_t.bitcast(F32R))
        nc.sync.dma_start(out=wo_sb, in_=wo_t.bitcast(F32R))

        qT = persist.tile([P, 2, S], F32R)               # [pair-cols, pair, seq]
        kT = persist.tile([P, 2, S], F32R)
        v_sb = persist.tile([P, NSB, GH, HD + 1], F32R)  # ones col appended
        oT = persist.tile([P, 2, S], F32R)
        nc.vector.tensor_copy(
            out=v_sb[:, :, :, HD:HD + 1],
            in_=ones_f32[:, 0:1].to_broadcast((P, NSB, GH, 1)))

        # ---- Phase A: projections, streamed over seq chunks ----
        with ExitStack() as ph_a:
            xpool = ph_a.enter_context(tc.tile_pool(name="xchunk", bufs=2))
            ps_qk = ph_a.enter_context(
                tc.tile_pool(name="ps_qk", bufs=4, space="PSUM"))
            ps_v = ph_a.enter_context(
                tc.tile_pool(name="ps_v", bufs=2, space="PSUM"))
            for ch in range(NCH):
                xt = xpool.tile([P, KD, CHW], F32R, tag="xt")
                nc.sync.dma_start(
                    out=xt,
                    in_=xT_t[:, :, ch * CHW:(ch + 1) * CHW].bitcast(F32R))
                for pair in range(2):
                    pq = ps_qk.tile([P, CHW], F32, tag="pqk")
                    pk = ps_qk.tile([P, CHW], F32, tag="pqk")
                    for k in range(KD):
                        st, sp = (k == 0), (k == KD - 1)
                        nc.tensor.matmul(
                            pq, wq_sb[:, k, pair * P:(pair + 1) * P],
                            xt[:, k, :], start=st, stop=sp)
                        nc.tensor.matmul(
                            pk, wk_sb[:, k, pair * P:(pair + 1) * P],
                            xt[:, k, :], start=st, stop=sp)
                    nc.vector.tensor_copy(
                        out=qT[:, pair, ch * CHW:(ch + 1) * CHW], in_=pq)
                    nc.vector.tensor_copy(
                        out=kT[:, pair, ch * CHW:(ch + 1) * CHW], in_=pk)
                for s4 in range(CHW // P):
                    sb = ch * (CHW // P) + s4
                    pv = ps_v.tile([P, GC], F32, tag="pv")
                    for k in range(KD):
                        nc.tensor.matmul(
                            pv, xt[:, k, s4 * P:(s4 + 1) * P],
                            wv_sb[:, k, :],
                            start=(k == 0), stop=(k == KD - 1))
                    nc.vector.tensor_copy(
                        out=v_sb[:, sb, :, 0:HD],
                        in_=pv[:].rearrange("p (h d) -> p h d", h=GH))

        # ---- Phase B: attention per head ----
        with ExitStack() as ph_b:
            ps_sc = ph_b.enter_context(
                tc.tile_pool(name="ps_sc", bufs=2, space="PSUM"))
            ps_pv = ph_b.enter_context(
                tc.tile_pool(name="ps_pv", bufs=1, space="PSUM"))
            ps_bc = ph_b.enter_context(
                tc.tile_pool(name="ps_bc", bufs=1, space="PSUM"))
            ppool = ph_b.enter_context(tc.tile_pool(name="pstrip", bufs=3))
            npool = ph_b.enter_context(tc.tile_pool(name="norm", bufs=4))

            for pair in range(2):
                for hp in range(2):
                    h = pair * 2 + hp
                    bp = hp * HD
                    for c in range(NCH):
                        njb = 4 * c + 4
                        pvacc = ps_pv.tile([HD + 1, CHW], F32, tag="pvacc")
                        jb0 = 0
                        while jb0 < njb:
                            w = min(3, njb - jb0)
                            sc = ps_sc.tile([P, 3, CHW], F32, tag="sc")
                            pt = ppool.tile([P, 3, CHW], F32R, tag="pt")
                            for t in range(w):
                                jb = jb0 + t
                                nc.tensor.matmul(
                                    sc[:, t, :],
                                    kT[bp:bp + HD, pair,
                                       jb * P:(jb + 1) * P],
                                    qT[bp:bp + HD, pair,
                                       c * CHW:(c + 1) * CHW])
                            nc.scalar.activation(
                                pt[:, :w, :], sc[:, :w, :],
                                mybir.ActivationFunctionType.Exp, scale=SCALE)
                            for t in range(w):
                                jb = jb0 + t
                                if jb >= 4 * c:          # diagonal square
                                    tl = jb - 4 * c
                                    if tl > 0:
                                        nc.vector.tensor_copy(
                                            out=pt[:, t, 0:tl * P],
                                            in_=zeros_f32[:, 0:tl * P])
                                    nc.vector.tensor_mul(
                                        pt[:, t, tl * P:(tl + 1) * P],
                                        pt[:, t, tl * P:(tl + 1) * P],
                                        trimask)
                            for t in range(w):
                                jb = jb0 + t
                                nc.tensor.matmul(
                                    pvacc, v_sb[:, jb, h, :],
                                    pt[:, t, :],
                                    start=(jb == 0), stop=(jb == njb - 1))
                            jb0 += w
                        # normalize: O^T = num * recip(denom) broadcast
                        rec = npool.tile([P, CHW], F32R, tag="rec")
                        with nc.allow_low_precision(
                                reason="f32r rounding of softmax denom"):
                            nc.vector.reciprocal(
                                out=rec[HD:HD + 1, :], in_=pvacc[HD:HD + 1, :])
                        bc = ps_bc.tile([HD, CHW], F32, tag="bc")
                        nc.tensor.matmul(
                            bc, ones_sb[HD:HD + 1, :], rec[HD:HD + 1, :])
                        onum = npool.tile([HD, CHW], F32, tag="onum")
                        nc.vector.tensor_copy(out=onum, in_=pvacc[0:HD, :])
                        if hp == 0:
                            nc.vector.tensor_mul(
                                oT[0:HD, pair, c * CHW:(c + 1) * CHW],
                                onum, bc)
                        else:
                            tmp = npool.tile([HD, CHW], F32R, tag="otmp")
                            nc.vector.tensor_mul(tmp, onum, bc)
                            nc.sync.dma_start(
                                out=oT[HD:P, pair, c * CHW:(c + 1) * CHW],
                                in_=tmp)

        # ---- Phase C: output projection ----
        with ExitStack() as ph_c:
            ps_y = ph_c.enter_context(
                tc.tile_pool(name="ps_y", bufs=4, space="PSUM"))
            ypool = ph_c.enter_context(tc.tile_pool(name="ystage", bufs=2))
            for sb in range(NSB):
                ysb = ypool.tile([P, D], F32, tag="ysb")
                for nch in range(2):
                    py = ps_y.tile([P, CHW], F32, tag="py")
                    for gc in range(2):
                        nc.tensor.matmul(
                            py, oT[:, gc, sb * P:(sb + 1) * P],
                            wo_sb[:, gc, nch * CHW:(nch + 1) * CHW],
                            start=(gc == 0), stop=(gc == 1))
                    nc.any.tensor_copy(
                        out=ysb[:, nch * CHW:(nch + 1) * CHW], in_=py)
                nc.sync.dma_start(out=y[sb * P:(sb + 1) * P, :], in_=ysb)


def _fix_matmul_waits(nc):
    """fp32r matmuls lower to a self-loading LDWEIGHTS struct that can carry
    only one sync wait. Hoist excess waits onto NoOps inserted immediately
    before the matmul in the scheduled stream (same engine, so program order
    preserves the wait semantics)."""
    fixed = 0
    for blk in nc.m.functions[0].blocks:
        insts = blk.instructions
        idx = 0
        while idx < len(insts):
            inst = insts[idx]
            if True:
                si = getattr(inst, "sync_info", None)
                if si is not None and len(si.on_wait) > 1:
                    waits = list(si.on_wait)
                    for j, wt in enumerate(waits[:-1]):
                        nop = mybir.InstNoOp(
                            name=f"I-wfix{fixed}-{j}-{inst.name}",
                            engine=inst.engine,
                            sync_info=mybir.SyncInfo(
                                on_wait=[wt], on_update=[]))
                        insts.insert(idx, nop)
                        idx += 1
                    inst.sync_info = mybir.SyncInfo(
                        on_wait=[waits[-1]], on_update=list(si.on_update))
                    fixed += 1
            idx += 1
    return fixed


def _build():
    global _NC_CACHE
    if _NC_CACHE is None:
        nc = bass.Bass()
        with tile.TileContext(nc) as tc:
            _emit(tc)
        _fix_matmul_waits(nc)
        _NC_CACHE = nc
    return _NC_CACHE


def kernel(x, Wq, Wkv, Wo):
    global LAST_RESULTS
    x = np.asarray(x, dtype=np.float32)
    Wq = np.asarray(Wq, dtype=np.float32)
    Wkv = np.asarray(Wkv, dtype=np.float32)
    Wo = np.asarray(Wo, dtype=np.float32)

    nc = _build()
    in_maps = []
    for c in range(8):
        b, g = divmod(c, 4)
        cs = slice(GC * g, GC * (g + 1))
        in_maps.append({
            "xT": np.ascontiguousarray(x[b].T),
            "wq": np.ascontiguousarray(Wq[:, cs]),
            "wk": np.ascontiguousarray(Wkv[:, 0:D][:, cs]),
            "wv": np.ascontiguousarray(Wkv[:, D:2 * D][:, cs]),
            "wo": np.ascontiguousarray(Wo[cs, :]),
        })

    trace = os.environ.get("ATTN_KERNEL_TRACE", "0") == "1"
    res = run_bass_kernel_spmd(nc, in_maps, list(range(8)), trace=trace)
    LAST_RESULTS = res

    out = np.zeros((B, S, D), dtype=np.float32)
    for c in range(8):
        b = c // 4
        out[b] += res.results[c]["y"]
    return out


if __name__ == "__main__":
    rng = np.random.default_rng(0)
    s = 1.0 / np.sqrt(D)
    inputs = {
        "x": rng.standard_normal((B, S, D), dtype=np.float32),
        "Wq": rng.standard_normal((D, D), dtype=np.float32) * s,
        "Wkv": rng.standard_normal((D, 2 * D), dtype=np.float32) * s,
        "Wo": rng.standard_normal((D, D), dtype=np.float32) * s,
    }
    out = kernel(**inputs)
    print("out", out.shape, out.dtype, float(np.abs(out).mean()))


# revision 9
# speedup vs baseline: 1.1520x; 1.1520x over previous
"""Multi-head causal attention (B=2, S=2048, D=1024, H=16) on 8 TRN2 NeuronCores.

Sharding: core c handles batch b = c//4 and head-group g = c%4 (4 heads, 256 dims).
Each core computes Q/K/V projections for its head group from x[b], runs causal
attention per head, and applies its 256 rows of Wo, producing a partial [S, D]
output. The host sums the 4 head-group partials per batch.

Device algorithm (per core); matmul operands bf16, accumulation fp32 in PSUM:
  qT/kT = Wq_g^T @ x^T, stored [64*2, pair, S] (head dims on partitions)
  v     = x @ Wv_g, stored per 128-seq block with an appended ones column
  per head, per 512-wide i-chunk:
    S^T[j,i] strips via matmul(lhsT=kT_block, rhs=qT_chunk)  (K=64, row-paired
    across the two heads of a pair via tile_position)
    P~^T = exp(scale * S^T)  (ScalarE, batched over 3 strips), causal-masked
    O'^T[65, i] += V'_j^T @ P~^T_j   (PSUM accumulate; row 64 = softmax denom)
    O^T = O'^T[0:64] * recip(O'^T[64])  (approx-recip + GpSimd row broadcast)
  y = O @ Wo_g (lhsT = O^T tiles), DMA out.
"""

import os

import ml_dtypes
import numpy as np

import concourse.bass as bass
import concourse.mybir as mybir
import concourse.tile as tile
from concourse.bass_utils import run_bass_kernel_spmd
from concourse.masks import make_upper_triangular

F32 = mybir.dt.float32
BF16 = mybir.dt.bfloat16

B, S, D, H = 2, 2048, 1024, 16
HD = 64                     # head dim
GH = 4                      # heads per core
GC = GH * HD                # 256 projection cols per core
P = 128
KD = D // P                 # 8 contraction chunks for projections
NSB = S // P                # 16 seq blocks
CHW = 512                   # i-chunk width
NCH = S // CHW              # 4 i-chunks
SCALE = HD ** -0.5

_NC_CACHE = None
LAST_RESULTS = None         # BassKernelResults of the most recent run (for test.py)


def _emit(tc):
    nc = tc.nc
    xT = nc.dram_tensor("xT", [D, S], BF16, kind="ExternalInput")
    wq = nc.dram_tensor("wq", [D, GC], BF16, kind="ExternalInput")
    wk = nc.dram_tensor("wk", [D, GC], BF16, kind="ExternalInput")
    wv = nc.dram_tensor("wv", [D, GC], BF16, kind="ExternalInput")
    wo = nc.dram_tensor("wo", [GC, D], BF16, kind="ExternalInput")
    y = nc.dram_tensor("y", [S, D], F32, kind="ExternalOutput")

    xT_t = xT[:].rearrange("(o p) s -> p o s", p=P)      # [128, 8, S]
    wq_t = wq[:].rearrange("(o p) c -> p o c", p=P)      # [128, 8, 256]
    wk_t = wk[:].rearrange("(o p) c -> p o c", p=P)
    wv_t = wv[:].rearrange("(o p) c -> p o c", p=P)
    wo_t = wo[:].rearrange("(o p) n -> p o n", p=P)      # [128, 2, 1024]

    from contextlib import ExitStack

    with ExitStack() as top:
        persist = top.enter_context(tc.tile_pool(name="persist", bufs=1))

        trimask = persist.tile([P, P], BF16)             # 1.0 where j<=i else 0
        make_upper_triangular(nc, trimask, val=1.0, diag=True)
        ones_bf = persist.tile([P, HD], BF16)
        nc.vector.memset(ones_bf, 1.0)
        zeros_bf = persist.tile([P, 3 * P], BF16)
        nc.vector.memset(zeros_bf, 0.0)

        wq_sb = persist.tile([P, KD, GC], BF16)
        wk_sb = persist.tile([P, KD, GC], BF16)
        wv_sb = persist.tile([P, KD, GC], BF16)
        wo_sb = persist.tile([P, 2, D], BF16)
        nc.sync.dma_start(out=wq_sb, in_=wq_t)
        nc.sync.dma_start(out=wk_sb, in_=wk_t)
        nc.sync.dma_start(out=wv_sb, in_=wv_t)
        nc.sync.dma_start(out=wo_sb, in_=wo_t)

        qT = persist.tile([P, 2, S], BF16)               # [pair-cols, pair, seq]
        kT = persist.tile([P, 2, S], BF16)
        v_sb = persist.tile([P, NSB, GH, HD + 1], BF16)  # ones col appended
        oT = persist.tile([P, 2, S], BF16)
        nc.vector.tensor_copy(
            out=v_sb[:, :, :, HD:HD + 1],
            in_=ones_bf[:, 0:1].to_broadcast((P, NSB, GH, 1)))

        # ---- Phase A: projections, streamed over seq chunks ----
        with ExitStack() as ph_a:
            xpool = ph_a.enter_context(tc.tile_pool(name="xchunk", bufs=2))
            ps_qk = ph_a.enter_context(
                tc.tile_pool(name="ps_qk", bufs=4, space="PSUM"))
            ps_v = ph_a.enter_context(
                tc.tile_pool(name="ps_v", bufs=2, space="PSUM"))
            for ch in range(NCH):
                xt = xpool.tile([P, KD, CHW], BF16, tag="xt")
                nc.sync.dma_start(
                    out=xt, in_=xT_t[:, :, ch * CHW:(ch + 1) * CHW])
                for pair in range(2):
                    pq = ps_qk.tile([P, CHW], F32, tag="pqk")
                    pk = ps_qk.tile([P, CHW], F32, tag="pqk")
                    for k in range(KD):
                        st, sp = (k == 0), (k == KD - 1)
                        nc.tensor.matmul(
                            pq, wq_sb[:, k, pair * P:(pair + 1) * P],
                            xt[:, k, :], start=st, stop=sp)
                        nc.tensor.matmul(
                            pk, wk_sb[:, k, pair * P:(pair + 1) * P],
                            xt[:, k, :], start=st, stop=sp)
                    nc.vector.tensor_copy(
                        out=qT[:, pair, ch * CHW:(ch + 1) * CHW], in_=pq)
                    nc.vector.tensor_copy(
                        out=kT[:, pair, ch * CHW:(ch + 1) * CHW], in_=pk)
                for s4 in range(CHW // P):
                    sb = ch * (CHW // P) + s4
                    pv = ps_v.tile([P, GC], F32, tag="pv")
                    for k in range(KD):
                        nc.tensor.matmul(
                            pv, xt[:, k, s4 * P:(s4 + 1) * P],
                            wv_sb[:, k, :],
                            start=(k == 0), stop=(k == KD - 1))
                    nc.vector.tensor_copy(
                        out=v_sb[:, sb, :, 0:HD],
                        in_=pv[:].rearrange("p (h d) -> p h d", h=GH))

        # ---- Phase B: attention per head ----
        with ExitStack() as ph_b:
            ps_sc = ph_b.enter_context(
                tc.tile_pool(name="ps_sc", bufs=2, space="PSUM"))
            ps_pv = ph_b.enter_context(
                tc.tile_pool(name="ps_pv", bufs=1, space="PSUM"))
            ps_bc = ph_b.enter_context(
                tc.tile_pool(name="ps_bc", bufs=1, space="PSUM"))
            ppool = ph_b.enter_context(tc.tile_pool(name="pstrip", bufs=3))
            npool = ph_b.enter_context(tc.tile_pool(name="norm", bufs=4))

            for pair in range(2):
                for hp in range(2):
                    h = pair * 2 + hp
                    bp = hp * HD
                    for c in range(NCH):
                        njb = 4 * c + 4
                        pvacc = ps_pv.tile([HD + 1, CHW], F32, tag="pvacc")
                        jb0 = 0
                        while jb0 < njb:
                            w = min(3, njb - jb0)
                            sc = ps_sc.tile([P, 3, CHW], F32, tag="sc")
                            pt = ppool.tile([P, 3, CHW], BF16, tag="pt")
                            for t in range(w):
                                jb = jb0 + t
                                nc.tensor.matmul(
                                    sc[:, t, :],
                                    kT[bp:bp + HD, pair,
                                       jb * P:(jb + 1) * P],
                                    qT[bp:bp + HD, pair,
                                       c * CHW:(c + 1) * CHW])
                            nc.scalar.activation(
                                pt[:, :w, :], sc[:, :w, :],
                                mybir.ActivationFunctionType.Exp, scale=SCALE)
                            for t in range(w):
                                jb = jb0 + t
                                if jb >= 4 * c:          # diagonal square
                                    tl = jb - 4 * c
                                    if tl > 0:
                                        nc.vector.tensor_copy(
                                            out=pt[:, t, 0:tl * P],
                                            in_=zeros_bf[:, 0:tl * P])
                                    nc.vector.tensor_mul(
                                        pt[:, t, tl * P:(tl + 1) * P],
                                        pt[:, t, tl * P:(tl + 1) * P],
                                        trimask)
                            for t in range(w):
                                jb = jb0 + t
                                nc.tensor.matmul(
                                    pvacc, v_sb[:, jb, h, :],
                                    pt[:, t, :],
                                    start=(jb == 0), stop=(jb == njb - 1))
                            jb0 += w
                        # normalize: O^T = num * recip(denom) broadcast
                        rec = npool.tile([P, CHW], BF16, tag="rec")
                        with nc.allow_low_precision(
                                reason="softmax denom recip rounded to bf16"):
                            nc.vector.reciprocal(
                                out=rec[HD:HD + 1, :],
                                in_=pvacc[HD:HD + 1, :])
                        bc = ps_bc.tile([HD, CHW], F32, tag="bc")
                        nc.tensor.matmul(
                            bc, ones_bf[HD:HD + 1, :], rec[HD:HD + 1, :])
                        bcr = npool.tile([HD, CHW], F32, tag="bcr")
                        nc.vector.tensor_copy(out=bcr, in_=bc)
                        if hp == 0:
                            nc.vector.tensor_mul(
                                oT[0:HD, pair, c * CHW:(c + 1) * CHW],
                                pvacc[0:HD, :], bcr)
                        else:
                            tmp = npool.tile([HD, CHW], BF16, tag="otmp")
                            nc.vector.tensor_mul(tmp, pvacc[0:HD, :], bcr)
                            nc.sync.dma_start(
                                out=oT[HD:P, pair, c * CHW:(c + 1) * CHW],
                                in_=tmp)

        # ---- Phase C: output projection ----
        with ExitStack() as ph_c:
            ps_y = ph_c.enter_context(
                tc.tile_pool(name="ps_y", bufs=4, space="PSUM"))
            ypool = ph_c.enter_context(tc.tile_pool(name="ystage", bufs=2))
            for sb in range(NSB):
                ysb = ypool.tile([P, D], F32, tag="ysb")
                for nch in range(2):
                    py = ps_y.tile([P, CHW], F32, tag="py")
                    for gc in range(2):
                        nc.tensor.matmul(
                            py, oT[:, gc, sb * P:(sb + 1) * P],
                            wo_sb[:, gc, nch * CHW:(nch + 1) * CHW],
                            start=(gc == 0), stop=(gc == 1))
                    nc.any.tensor_copy(
                        out=ysb[:, nch * CHW:(nch + 1) * CHW], in_=py)
                nc.sync.dma_start(out=y[sb * P:(sb + 1) * P, :], in_=ysb)


def _fix_instruction_waits(nc):
    """Some lowered ISA structs (fp32r matmul LDW, DMA pseudo) carry at most
    one sync wait. Normalize: hoist excess waits onto NoOps inserted
    immediately before the instruction in the scheduled stream (same engine,
    so program order preserves the wait semantics)."""
    fixed = 0
    for blk in nc.m.functions[0].blocks:
        insts = blk.instructions
        idx = 0
        while idx < len(insts):
            inst = insts[idx]
            si = getattr(inst, "sync_info", None)
            if si is not None and len(si.on_wait) > 1:
                waits = list(si.on_wait)
                for j, wt in enumerate(waits[:-1]):
                    nop = mybir.InstNoOp(
                        name=f"I-wfix{fixed}-{j}-{inst.name}",
                        engine=inst.engine,
                        sync_info=mybir.SyncInfo(on_wait=[wt], on_update=[]))
                    insts.insert(idx, nop)
                    idx += 1
                inst.sync_info = mybir.SyncInfo(
                    on_wait=[waits[-1]], on_update=list(si.on_update))
                fixed += 1
            idx += 1
    return fixed


def _build():
    global _NC_CACHE
    if _NC_CACHE is None:
        nc = bass.Bass()
        with tile.TileContext(nc) as tc:
            _emit(tc)
        _fix_instruction_waits(nc)
        _NC_CACHE = nc
    return _NC_CACHE


def kernel(x, Wq, Wkv, Wo):
    global LAST_RESULTS
    x = np.asarray(x, dtype=np.float32)
    Wq = np.asarray(Wq, dtype=np.float32)
    Wkv = np.asarray(Wkv, dtype=np.float32)
    Wo = np.asarray(Wo, dtype=np.float32)

    nc = _build()
    bf = ml_dtypes.bfloat16
    in_maps = []
    for c in range(8):
        b, g = divmod(c, 4)
        cs = slice(GC * g, GC * (g + 1))
        in_maps.append({
            "xT": np.ascontiguousarray(x[b].T).astype(bf),
            "wq": np.ascontiguousarray(Wq[:, cs]).astype(bf),
            "wk": np.ascontiguousarray(Wkv[:, 0:D][:, cs]).astype(bf),
            "wv": np.ascontiguousarray(Wkv[:, D:2 * D][:, cs]).astype(bf),
            "wo": np.ascontiguousarray(Wo[cs, :]).astype(bf),
        })

    trace = os.environ.get("ATTN_KERNEL_TRACE", "0") == "1"
    res = run_bass_kernel_spmd(nc, in_maps, list(range(8)), trace=trace)
    LAST_RESULTS = res

    out = np.zeros((B, S, D), dtype=np.float32)
    for c in range(8):
        b = c // 4
        out[b] += res.results[c]["y"]
    return out


if __name__ == "__main__":
    rng = np.random.default_rng(0)
    s = 1.0 / np.sqrt(D)
    inputs = {
        "x": rng.standard_normal((B, S, D), dtype=np.float32),
        "Wq": rng.standard_normal((D, D), dtype=np.float32) * s,
        "Wkv": rng.standard_normal((D, 2 * D), dtype=np.float32) * s,
        "Wo": rng.standard_normal((D, D), dtype=np.float32) * s,
    }
    out = kernel(**inputs)
    print("out", out.shape, out.dtype, float(np.abs(out).mean()))


# revision 10
# speedup vs baseline: 1.2480x; 1.0834x over previous
"""Multi-head causal attention (B=2, S=2048, D=1024, H=16) on 8 TRN2 NeuronCores.

Sharding: core c handles batch b = c//4 and head-group g = c%4 (4 heads, 256 dims).
Each core computes Q/K/V projections for its head group from x[b], runs causal
attention per head, and applies its 256 rows of Wo, producing a partial [S, D]
output. The host sums the 4 head-group partials per batch.

Device algorithm (per core); matmul operands bf16, accumulation fp32 in PSUM:
  qT/kT = Wq_g^T @ x^T, stored [64*2, pair, S] (head dims on partitions)
  v     = x @ Wv_g, stored per 128-seq block with an appended ones column
  per head, per 512-wide i-chunk:
    S^T[j,i] strips via matmul(lhsT=kT_block, rhs=qT_chunk)  (K=64, row-paired
    across the two heads of a pair via tile_position)
    P~^T = exp(scale * S^T)  (ScalarE, batched over 3 strips), causal-masked
    O'^T[65, i] += V'_j^T @ P~^T_j   (PSUM accumulate; row 64 = softmax denom)
    O^T = O'^T[0:64] * recip(O'^T[64])  (approx-recip + GpSimd row broadcast)
  y = O @ Wo_g (lhsT = O^T tiles), DMA out.
"""

import os

import ml_dtypes
import numpy as np

import concourse.bass as bass
import concourse.mybir as mybir
import concourse.tile as tile
from concourse.bass_utils import run_bass_kernel_spmd
from concourse.masks import make_upper_triangular

F32 = mybir.dt.float32
BF16 = mybir.dt.bfloat16

B, S, D, H = 2, 2048, 1024, 16
HD = 64                     # head dim
GH = 4                      # heads per core
GC = GH * HD                # 256 projection cols per core
P = 128
KD = D // P                 # 8 contraction chunks for projections
NSB = S // P                # 16 seq blocks
CHW = 512                   # i-chunk width
NCH = S // CHW              # 4 i-chunks
SCALE = HD ** -0.5

_NC_CACHE = None
LAST_RESULTS = None         # BassKernelResults of the most recent run (for test.py)


def _emit(tc):
    nc = tc.nc
    xT = nc.dram_tensor("xT", [D, S], BF16, kind="ExternalInput")
    wq = nc.dram_tensor("wq", [D, GC], BF16, kind="ExternalInput")
    wk = nc.dram_tensor("wk", [D, GC], BF16, kind="ExternalInput")
    wv = nc.dram_tensor("wv", [D, GC], BF16, kind="ExternalInput")
    wo = nc.dram_tensor("wo", [GC, D], BF16, kind="ExternalInput")
    y = nc.dram_tensor("y", [S, D], F32, kind="ExternalOutput")

    xT_t = xT[:].rearrange("(o p) s -> p o s", p=P)      # [128, 8, S]
    wq_t = wq[:].rearrange("(o p) c -> p o c", p=P)      # [128, 8, 256]
    wk_t = wk[:].rearrange("(o p) c -> p o c", p=P)
    wv_t = wv[:].rearrange("(o p) c -> p o c", p=P)
    wo_t = wo[:].rearrange("(o p) n -> p o n", p=P)      # [128, 2, 1024]

    from contextlib import ExitStack

    with ExitStack() as top:
        persist = top.enter_context(tc.tile_pool(name="persist", bufs=1))

        trimask = persist.tile([P, P], BF16)             # 1.0 where j<=i else 0
        make_upper_triangular(nc, trimask, val=1.0, diag=True)
        ones_bf = persist.tile([P, HD], BF16)
        nc.vector.memset(ones_bf, 1.0)
        zeros_bf = persist.tile([P, 3 * P], BF16)
        nc.vector.memset(zeros_bf, 0.0)

        wq_sb = persist.tile([P, KD, GC], BF16)
        wk_sb = persist.tile([P, KD, GC], BF16)
        wv_sb = persist.tile([P, KD, GC], BF16)
        wo_sb = persist.tile([P, 2, D], BF16)
        nc.sync.dma_start(out=wq_sb, in_=wq_t)
        nc.sync.dma_start(out=wk_sb, in_=wk_t)
        nc.sync.dma_start(out=wv_sb, in_=wv_t)
        nc.sync.dma_start(out=wo_sb, in_=wo_t)

        qT = persist.tile([P, 2, S], BF16)               # [pair-cols, pair, seq]
        kT = persist.tile([P, 2, S], BF16)
        v_sb = persist.tile([P, NSB, GH, HD + 1], BF16)  # ones col appended
        oT = persist.tile([P, 2, S], BF16)
        nc.vector.tensor_copy(
            out=v_sb[:, :, :, HD:HD + 1],
            in_=ones_bf[:, 0:1].to_broadcast((P, NSB, GH, 1)))

        # ---- Phase A: projections, streamed over seq chunks ----
        with ExitStack() as ph_a:
            xpool = ph_a.enter_context(tc.tile_pool(name="xchunk", bufs=2))
            ps_qk = ph_a.enter_context(
                tc.tile_pool(name="ps_qk", bufs=4, space="PSUM"))
            ps_v = ph_a.enter_context(
                tc.tile_pool(name="ps_v", bufs=2, space="PSUM"))
            for ch in range(NCH):
                xt = xpool.tile([P, KD, CHW], BF16, tag="xt")
                nc.sync.dma_start(
                    out=xt, in_=xT_t[:, :, ch * CHW:(ch + 1) * CHW])
                for pair in range(2):
                    pq = ps_qk.tile([P, CHW], F32, tag="pqk")
                    pk = ps_qk.tile([P, CHW], F32, tag="pqk")
                    for k in range(KD):
                        st, sp = (k == 0), (k == KD - 1)
                        nc.tensor.matmul(
                            pq, wq_sb[:, k, pair * P:(pair + 1) * P],
                            xt[:, k, :], start=st, stop=sp)
                        nc.tensor.matmul(
                            pk, wk_sb[:, k, pair * P:(pair + 1) * P],
                            xt[:, k, :], start=st, stop=sp)
                    nc.vector.tensor_copy(
                        out=qT[:, pair, ch * CHW:(ch + 1) * CHW], in_=pq)
                    nc.vector.tensor_copy(
                        out=kT[:, pair, ch * CHW:(ch + 1) * CHW], in_=pk)
                for s4 in range(CHW // P):
                    sb = ch * (CHW // P) + s4
                    pv = ps_v.tile([P, GC], F32, tag="pv")
                    for k in range(KD):
                        nc.tensor.matmul(
                            pv, xt[:, k, s4 * P:(s4 + 1) * P],
                            wv_sb[:, k, :],
                            start=(k == 0), stop=(k == KD - 1))
                    nc.vector.tensor_copy(
                        out=v_sb[:, sb, :, 0:HD],
                        in_=pv[:].rearrange("p (h d) -> p h d", h=GH))

        # ---- Phase B: attention per head ----
        with ExitStack() as ph_b:
            ps_sc = ph_b.enter_context(
                tc.tile_pool(name="ps_sc", bufs=2, space="PSUM"))
            ps_pv = ph_b.enter_context(
                tc.tile_pool(name="ps_pv", bufs=2, space="PSUM"))
            dpool = ph_b.enter_context(
                tc.tile_pool(name="dscr", bufs=4, space="DRAM"))
            ppool = ph_b.enter_context(tc.tile_pool(name="pstrip", bufs=3))
            npool = ph_b.enter_context(tc.tile_pool(name="norm", bufs=4))

            for pair in range(2):
                for hp in range(2):
                    h = pair * 2 + hp
                    bp = hp * HD
                    for c in range(NCH):
                        njb = 4 * c + 4
                        pvacc = ps_pv.tile([HD + 1, CHW], F32, tag="pvacc")
                        jb0 = 0
                        while jb0 < njb:
                            w = min(3, njb - jb0)
                            sc = ps_sc.tile([P, 3, CHW], F32, tag="sc")
                            pt = ppool.tile([P, 3, CHW], BF16, tag="pt")
                            for t in range(w):
                                jb = jb0 + t
                                nc.tensor.matmul(
                                    sc[:, t, :],
                                    kT[bp:bp + HD, pair,
                                       jb * P:(jb + 1) * P],
                                    qT[bp:bp + HD, pair,
                                       c * CHW:(c + 1) * CHW])
                            nc.scalar.activation(
                                pt[:, :w, :], sc[:, :w, :],
                                mybir.ActivationFunctionType.Exp, scale=SCALE)
                            for t in range(w):
                                jb = jb0 + t
                                if jb >= 4 * c:          # diagonal square
                                    tl = jb - 4 * c
                                    if tl > 0:
                                        nc.gpsimd.memset(
                                            pt[:, t, 0:tl * P], 0.0)
                                    nc.vector.tensor_mul(
                                        pt[:, t, tl * P:(tl + 1) * P],
                                        pt[:, t, tl * P:(tl + 1) * P],
                                        trimask)
                            for t in range(w):
                                jb = jb0 + t
                                nc.tensor.matmul(
                                    pvacc, v_sb[:, jb, h, :],
                                    pt[:, t, :],
                                    start=(jb == 0), stop=(jb == njb - 1))
                            jb0 += w
                        # normalize: O^T = num * recip(denom) broadcast
                        rec = npool.tile([P, CHW], F32, tag="rec")
                        nc.vector.reciprocal(
                            out=rec[HD:HD + 1, :], in_=pvacc[HD:HD + 1, :])
                        dscr = dpool.tile([1, CHW], F32, tag="dscr")
                        nc.sync.dma_start(out=dscr, in_=rec[HD:HD + 1, :])
                        bcr = npool.tile([HD, CHW], F32, tag="bcr")
                        dsrc = bass.AP(
                            tensor=dscr.tensor, offset=dscr.offset,
                            ap=[[0, HD]] + list(dscr.ap[1:]))
                        nc.sync.dma_start(out=bcr, in_=dsrc)
                        if hp == 0:
                            nc.vector.tensor_mul(
                                oT[0:HD, pair, c * CHW:(c + 1) * CHW],
                                pvacc[0:HD, :], bcr)
                        else:
                            tmp = npool.tile([HD, CHW], BF16, tag="otmp")
                            nc.vector.tensor_mul(tmp, pvacc[0:HD, :], bcr)
                            nc.sync.dma_start(
                                out=oT[HD:P, pair, c * CHW:(c + 1) * CHW],
                                in_=tmp)

        # ---- Phase C: output projection ----
        with ExitStack() as ph_c:
            ps_y = ph_c.enter_context(
                tc.tile_pool(name="ps_y", bufs=4, space="PSUM"))
            ypool = ph_c.enter_context(tc.tile_pool(name="ystage", bufs=2))
            for sb in range(NSB):
                ysb = ypool.tile([P, D], F32, tag="ysb")
                for nch in range(2):
                    py = ps_y.tile([P, CHW], F32, tag="py")
                    for gc in range(2):
                        nc.tensor.matmul(
                            py, oT[:, gc, sb * P:(sb + 1) * P],
                            wo_sb[:, gc, nch * CHW:(nch + 1) * CHW],
                            start=(gc == 0), stop=(gc == 1))
                    nc.any.tensor_copy(
                        out=ysb[:, nch * CHW:(nch + 1) * CHW], in_=py)
                nc.sync.dma_start(out=y[sb * P:(sb + 1) * P, :], in_=ysb)


def _fix_instruction_waits(nc):
    """Some lowered ISA structs (fp32r matmul LDW, DMA pseudo) carry at most
    one sync wait. Normalize: hoist excess waits onto NoOps inserted
    immediately before the instruction in the scheduled stream (same engine,
    so program order preserves the wait semantics)."""
    fixed = 0
    for blk in nc.m.functions[0].blocks:
        insts = blk.instructions
        idx = 0
        while idx < len(insts):
            inst = insts[idx]
            si = getattr(inst, "sync_info", None)
            if si is not None and len(si.on_wait) > 1:
                waits = list(si.on_wait)
                for j, wt in enumerate(waits[:-1]):
                    nop = mybir.InstNoOp(
                        name=f"I-wfix{fixed}-{j}-{inst.name}",
                        engine=inst.engine,
                        sync_info=mybir.SyncInfo(on_wait=[wt], on_update=[]))
                    insts.insert(idx, nop)
                    idx += 1
                inst.sync_info = mybir.SyncInfo(
                    on_wait=[waits[-1]], on_update=list(si.on_update))
                fixed += 1
            idx += 1
    return fixed


def _build():
    global _NC_CACHE
    if _NC_CACHE is None:
        nc = bass.Bass()
        with tile.TileContext(nc) as tc:
            _emit(tc)
        _fix_instruction_waits(nc)
        _NC_CACHE = nc
    return _NC_CACHE


def kernel(x, Wq, Wkv, Wo):
    global LAST_RESULTS
    x = np.asarray(x, dtype=np.float32)
    Wq = np.asarray(Wq, dtype=np.float32)
    Wkv = np.asarray(Wkv, dtype=np.float32)
    Wo = np.asarray(Wo, dtype=np.float32)

    nc = _build()
    bf = ml_dtypes.bfloat16
    in_maps = []
    for c in range(8):
        b, g = divmod(c, 4)
        cs = slice(GC * g, GC * (g + 1))
        in_maps.append({
            "xT": np.ascontiguousarray(x[b].T).astype(bf),
            "wq": np.ascontiguousarray(Wq[:, cs]).astype(bf),
            "wk": np.ascontiguousarray(Wkv[:, 0:D][:, cs]).astype(bf),
            "wv": np.ascontiguousarray(Wkv[:, D:2 * D][:, cs]).astype(bf),
            "wo": np.ascontiguousarray(Wo[cs, :]).astype(bf),
        })

    trace = os.environ.get("ATTN_KERNEL_TRACE", "0") == "1"
    res = run_bass_kernel_spmd(nc, in_maps, list(range(8)), trace=trace)
    LAST_RESULTS = res

    out = np.zeros((B, S, D), dtype=np.float32)
    for c in range(8):
        b = c // 4
        out[b] += res.results[c]["y"]
    return out


if __name__ == "__main__":
    rng = np.random.default_rng(0)
    s = 1.0 / np.sqrt(D)
    inputs = {
        "x": rng.standard_normal((B, S, D), dtype=np.float32),
        "Wq": rng.standard_normal((D, D), dtype=np.float32) * s,
        "Wkv": rng.standard_normal((D, 2 * D), dtype=np.float32) * s,
        "Wo": rng.standard_normal((D, D), dtype=np.float32) * s,
    }
    out = kernel(**inputs)
    print("out", out.shape, out.dtype, float(np.abs(out).mean()))


# revision 11
# speedup vs baseline: 1.2578x; 1.0078x over previous
"""Multi-head causal attention (B=2, S=2048, D=1024, H=16) on 8 TRN2 NeuronCores.

Sharding: core c handles batch b = c//4 and head-group g = c%4 (4 heads, 256 dims).
Each core computes Q/K/V projections for its head group from x[b], runs causal
attention per head, and applies its 256 rows of Wo, producing a partial [S, D]
output. The host sums the 4 head-group partials per batch.

Device algorithm (per core); matmul operands bf16, accumulation fp32 in PSUM:
  qT/kT = Wq_g^T @ x^T, stored [64*2, pair, S] (head dims on partitions)
  v     = x @ Wv_g, stored per 128-seq block with an appended ones column
  per head, per 512-wide i-chunk (processed widest-first):
    S^T[j,i] strips via matmul(lhsT=kT_block, rhs=qT_chunk), diagonal strips
    narrowed to their causally-valid column range
    P~^T = exp(scale * S^T)  (ScalarE, batched over 3 strips), diagonal blocks
    masked with an upper-triangular 0/1 multiply
    O'^T[65, i] += V'_j^T @ P~^T_j   (PSUM accumulate; row 64 = softmax denom)
    O^T = O'^T[0:64] * recip(O'^T[64])  (denominator reciprocal broadcast
    across partitions via a DRAM-bounce DMA)
  y = O @ Wo_g (lhsT = O^T tiles), DMA out.

Pair-1 Q/K projections are emitted between pair-0 and pair-1 attention so the
TensorE fills the ScalarE-bound exp window.
"""

import os

import ml_dtypes
import numpy as np

import concourse.bass as bass
import concourse.mybir as mybir
import concourse.tile as tile
from concourse.bass_utils import run_bass_kernel_spmd
from concourse.masks import make_upper_triangular

F32 = mybir.dt.float32
BF16 = mybir.dt.bfloat16

B, S, D, H = 2, 2048, 1024, 16
HD = 64                     # head dim
GH = 4                      # heads per core
GC = GH * HD                # 256 projection cols per core
P = 128
KD = D // P                 # 8 contraction chunks for projections
NSB = S // P                # 16 seq blocks
CHW = 512                   # i-chunk width
NCH = S // CHW              # 4 i-chunks
SCALE = HD ** -0.5

_NC_CACHE = None
LAST_RESULTS = None         # BassKernelResults of the most recent run (for test.py)


def _emit_pair_attention(tc, pair, pools, tensors):
    nc = tc.nc
    ps_sc, ps_pv, dpool, ppool, npool = pools
    qT, kT, v_sb, oT, trimask = tensors
    for hp in range(2):
        h = pair * 2 + hp
        bp = hp * HD
        for c in range(NCH - 1, -1, -1):      # widest chunk first
            njb = 4 * c + 4
            pvacc = ps_pv.tile([HD + 1, CHW], F32, tag="pvacc")
            jb0 = 0
            while jb0 < njb:
                w = min(3, njb - jb0)
                sc = ps_sc.tile([P, 3, CHW], F32, tag="sc")
                pt = ppool.tile([P, 3, CHW], BF16, tag="pt")
                for t in range(w):
                    jb = jb0 + t
                    tl = max(0, jb - 4 * c) * P
                    nc.tensor.matmul(
                        sc[:, t, tl:],
                        kT[bp:bp + HD, pair, jb * P:(jb + 1) * P],
                        qT[bp:bp + HD, pair, c * CHW + tl:(c + 1) * CHW])
                nc.scalar.activation(
                    pt[:, :w, :], sc[:, :w, :],
                    mybir.ActivationFunctionType.Exp, scale=SCALE)
                for t in range(w):
                    jb = jb0 + t
                    if jb >= 4 * c:           # diagonal block: causal mask
                        tl = (jb - 4 * c) * P
                        nc.vector.tensor_mul(
                            pt[:, t, tl:tl + P], pt[:, t, tl:tl + P],
                            trimask)
                for t in range(w):
                    jb = jb0 + t
                    tl = max(0, jb - 4 * c) * P
                    nc.tensor.matmul(
                        pvacc[:, tl:], v_sb[:, jb, h, :], pt[:, t, tl:],
                        start=(jb == 0), stop=(jb == njb - 1))
                jb0 += w
            # normalize: O^T = num * recip(denom), denom broadcast across
            # partitions by bouncing the 2KB recip row through DRAM
            rec = npool.tile([P, CHW], F32, tag="rec")
            nc.vector.reciprocal(
                out=rec[HD:HD + 1, :], in_=pvacc[HD:HD + 1, :])
            dscr = dpool.tile([1, CHW], F32, tag="dscr")
            nc.sync.dma_start(out=dscr, in_=rec[HD:HD + 1, :])
            bcr = npool.tile([HD, CHW], F32, tag="bcr")
            dsrc = bass.AP(
                tensor=dscr.tensor, offset=dscr.offset,
                ap=[[0, HD]] + list(dscr.ap[1:]))
            nc.sync.dma_start(out=bcr, in_=dsrc)
            if hp == 0:
                nc.vector.tensor_mul(
                    oT[0:HD, pair, c * CHW:(c + 1) * CHW],
                    pvacc[0:HD, :], bcr)
            else:
                tmp = npool.tile([HD, CHW], BF16, tag="otmp")
                nc.vector.tensor_mul(tmp, pvacc[0:HD, :], bcr)
                nc.sync.dma_start(
                    out=oT[HD:P, pair, c * CHW:(c + 1) * CHW], in_=tmp)


def _emit(tc):
    nc = tc.nc
    xT = nc.dram_tensor("xT", [D, S], BF16, kind="ExternalInput")
    wq = nc.dram_tensor("wq", [D, GC], BF16, kind="ExternalInput")
    wk = nc.dram_tensor("wk", [D, GC], BF16, kind="ExternalInput")
    wv = nc.dram_tensor("wv", [D, GC], BF16, kind="ExternalInput")
    wo = nc.dram_tensor("wo", [GC, D], BF16, kind="ExternalInput")
    y = nc.dram_tensor("y", [S, D], F32, kind="ExternalOutput")

    xT_t = xT[:].rearrange("(o p) s -> p o s", p=P)      # [128, 8, S]
    wq_t = wq[:].rearrange("(o p) c -> p o c", p=P)      # [128, 8, 256]
    wk_t = wk[:].rearrange("(o p) c -> p o c", p=P)
    wv_t = wv[:].rearrange("(o p) c -> p o c", p=P)
    wo_t = wo[:].rearrange("(o p) n -> p o n", p=P)      # [128, 2, 1024]

    from contextlib import ExitStack

    with ExitStack() as top:
        persist = top.enter_context(tc.tile_pool(name="persist", bufs=1))

        trimask = persist.tile([P, P], BF16)             # 1.0 where j<=i else 0
        make_upper_triangular(nc, trimask, val=1.0, diag=True)
        ones_bf = persist.tile([P, 1], BF16)
        nc.vector.memset(ones_bf, 1.0)

        wq_sb = persist.tile([P, KD, GC], BF16)
        wk_sb = persist.tile([P, KD, GC], BF16)
        wv_sb = persist.tile([P, KD, GC], BF16)
        wo_sb = persist.tile([P, 2, D], BF16)
        xfull = persist.tile([P, KD, S], BF16)
        # first-needed tensors first, split across the two HWDGE engines
        nc.scalar.dma_start(out=wq_sb, in_=wq_t)
        nc.sync.dma_start(out=wk_sb, in_=wk_t)
        for ch in range(NCH):
            eng = nc.sync if ch % 2 == 0 else nc.scalar
            eng.dma_start(
                out=xfull[:, :, ch * CHW:(ch + 1) * CHW],
                in_=xT_t[:, :, ch * CHW:(ch + 1) * CHW])
        nc.scalar.dma_start(out=wv_sb, in_=wv_t)
        nc.sync.dma_start(out=wo_sb, in_=wo_t)

        qT = persist.tile([P, 2, S], BF16)               # [pair-cols, pair, seq]
        kT = persist.tile([P, 2, S], BF16)
        v_sb = persist.tile([P, NSB, GH, HD + 1], BF16)  # ones col appended
        oT = persist.tile([P, 2, S], BF16)
        nc.vector.tensor_copy(
            out=v_sb[:, :, :, HD:HD + 1],
            in_=ones_bf[:, 0:1].to_broadcast((P, NSB, GH, 1)))

        tensors = (qT, kT, v_sb, oT, trimask)

        for pair in range(2):
            # ---- projections for this pair (V for both pairs on pair 0) ----
            with ExitStack() as ph_a:
                ps_qk = ph_a.enter_context(
                    tc.tile_pool(name=f"ps_qk{pair}", bufs=4, space="PSUM"))
                for ch in range(NCH):
                    pq = ps_qk.tile([P, CHW], F32, tag="pqk")
                    pk = ps_qk.tile([P, CHW], F32, tag="pqk")
                    for k in range(KD):
                        st, sp = (k == 0), (k == KD - 1)
                        nc.tensor.matmul(
                            pq, wq_sb[:, k, pair * P:(pair + 1) * P],
                            xfull[:, k, ch * CHW:(ch + 1) * CHW],
                            start=st, stop=sp)
                        nc.tensor.matmul(
                            pk, wk_sb[:, k, pair * P:(pair + 1) * P],
                            xfull[:, k, ch * CHW:(ch + 1) * CHW],
                            start=st, stop=sp)
                    nc.vector.tensor_copy(
                        out=qT[:, pair, ch * CHW:(ch + 1) * CHW], in_=pq)
                    nc.vector.tensor_copy(
                        out=kT[:, pair, ch * CHW:(ch + 1) * CHW], in_=pk)
                if pair == 0:
                    with tc.tile_pool(
                            name="ps_v", bufs=2, space="PSUM") as ps_v:
                        for sb in range(NSB):
                            pv = ps_v.tile([P, GC], F32, tag="pv")
                            for k in range(KD):
                                nc.tensor.matmul(
                                    pv, xfull[:, k, sb * P:(sb + 1) * P],
                                    wv_sb[:, k, :],
                                    start=(k == 0), stop=(k == KD - 1))
                            nc.vector.tensor_copy(
                                out=v_sb[:, sb, :, 0:HD],
                                in_=pv[:].rearrange("p (h d) -> p h d", h=GH))

            # ---- attention for this pair ----
            with ExitStack() as ph_b:
                ps_sc = ph_b.enter_context(
                    tc.tile_pool(name=f"ps_sc{pair}", bufs=2, space="PSUM"))
                ps_pv = ph_b.enter_context(
                    tc.tile_pool(name=f"ps_pv{pair}", bufs=2, space="PSUM"))
                dpool = ph_b.enter_context(
                    tc.tile_pool(name=f"dscr{pair}", bufs=4, space="DRAM"))
                ppool = ph_b.enter_context(
                    tc.tile_pool(name=f"pstrip{pair}", bufs=3))
                npool = ph_b.enter_context(
                    tc.tile_pool(name=f"norm{pair}", bufs=4))
                _emit_pair_attention(
                    tc, pair, (ps_sc, ps_pv, dpool, ppool, npool), tensors)

        # ---- output projection ----
        with ExitStack() as ph_c:
            ps_y = ph_c.enter_context(
                tc.tile_pool(name="ps_y", bufs=4, space="PSUM"))
            ypool = ph_c.enter_context(tc.tile_pool(name="ystage", bufs=2))
            for sb in range(NSB):
                ysb = ypool.tile([P, D], F32, tag="ysb")
                for nch in range(2):
                    py = ps_y.tile([P, CHW], F32, tag="py")
                    for gc in range(2):
                        nc.tensor.matmul(
                            py, oT[:, gc, sb * P:(sb + 1) * P],
                            wo_sb[:, gc, nch * CHW:(nch + 1) * CHW],
                            start=(gc == 0), stop=(gc == 1))
                    nc.vector.tensor_copy(
                        out=ysb[:, nch * CHW:(nch + 1) * CHW], in_=py)
                nc.sync.dma_start(out=y[sb * P:(sb + 1) * P, :], in_=ysb)


def _fix_instruction_waits(nc):
    """Some lowered ISA structs (fp32r matmul LDW, DMA pseudo) carry at most
    one sync wait. Normalize: hoist excess waits onto NoOps inserted
    immediately before the instruction in the scheduled stream (same engine,
    so program order preserves the wait semantics)."""
    fixed = 0
    for blk in nc.m.functions[0].blocks:
        insts = blk.instructions
        idx = 0
        while idx < len(insts):
            inst = insts[idx]
            si = getattr(inst, "sync_info", None)
            if si is not None and len(si.on_wait) > 1:
                waits = list(si.on_wait)
                for j, wt in enumerate(waits[:-1]):
                    nop = mybir.InstNoOp(
                        name=f"I-wfix{fixed}-{j}-{inst.name}",
                        engine=inst.engine,
                        sync_info=mybir.SyncInfo(on_wait=[wt], on_update=[]))
                    insts.insert(idx, nop)
                    idx += 1
                inst.sync_info = mybir.SyncInfo(
                    on_wait=[waits[-1]], on_update=list(si.on_update))
                fixed += 1
            idx += 1
    return fixed


def _build():
    global _NC_CACHE
    if _NC_CACHE is None:
        nc = bass.Bass()
        with tile.TileContext(nc) as tc:
            _emit(tc)
        _fix_instruction_waits(nc)
        _NC_CACHE = nc
    return _NC_CACHE


def kernel(x, Wq, Wkv, Wo):
    global LAST_RESULTS
    x = np.asarray(x, dtype=np.float32)
    Wq = np.asarray(Wq, dtype=np.float32)
    Wkv = np.asarray(Wkv, dtype=np.float32)
    Wo = np.asarray(Wo, dtype=np.float32)

    nc = _build()
    bf = ml_dtypes.bfloat16
    in_maps = []
    for c in range(8):
        b, g = divmod(c, 4)
        cs = slice(GC * g, GC * (g + 1))
        in_maps.append({
            "xT": np.ascontiguousarray(x[b].T).astype(bf),
            "wq": np.ascontiguousarray(Wq[:, cs]).astype(bf),
            "wk": np.ascontiguousarray(Wkv[:, 0:D][:, cs]).astype(bf),
            "wv": np.ascontiguousarray(Wkv[:, D:2 * D][:, cs]).astype(bf),
            "wo": np.ascontiguousarray(Wo[cs, :]).astype(bf),
        })

    trace = os.environ.get("ATTN_KERNEL_TRACE", "0") == "1"
    res = run_bass_kernel_spmd(nc, in_maps, list(range(8)), trace=trace)
    LAST_RESULTS = res

    out = np.zeros((B, S, D), dtype=np.float32)
    for c in range(8):
        b = c // 4
        out[b] += res.results[c]["y"]
    return out


if __name__ == "__main__":
    rng = np.random.default_rng(0)
    s = 1.0 / np.sqrt(D)
    inputs = {
        "x": rng.standard_normal((B, S, D), dtype=np.float32),
        "Wq": rng.standard_normal((D, D), dtype=np.float32) * s,
        "Wkv": rng.standard_normal((D, 2 * D), dtype=np.float32) * s,
        "Wo": rng.standard_normal((D, D), dtype=np.float32) * s,
    }
    out = kernel(**inputs)
    print("out", out.shape, out.dtype, float(np.abs(out).mean()))
